# revision 9
# baseline (speedup 1.0000x reference)
"""Steerable 3D conv block (nn_Block_66795331387589) on 8 Trainium2 NeuronCores.

Pipeline per core (data-parallel over batch x D-slabs, halo sliced on host):
  1. host: tensor-square channels, channel permutation, kernel assembly
     (basis x weights einsum), per-core D-slab slicing with 3-voxel halo.
  2. device: BN max-statistics (field norms via selector matmul), 8-core
     AllReduce(max), per-block scale vector via indicator matmul, scale the
     bf16 activation tiles in place, then the 7x7x7 conv as 343-tap
     shift-and-accumulate bf16 matmuls into PSUM, bias+ReLU on the l=0
     output channels, DMA out.
"""
import sys

sys.path.insert(0, "/opt/trn_rl_repo")

from contextlib import ExitStack

import ml_dtypes
import numpy as np

import concourse.bass as bass
import concourse.tile as tile
from concourse import bacc, mybir
from concourse.bass_utils import run_bass_kernel_spmd

N_CORES = 8
B, S = 2, 32
C1, C2 = 128, 100          # contract chunks over the 228 permuted channels
CIN = 228
COUT = 84
K = 7
PAD = S + 2 * 3            # 38
NP = 14                    # 8 owned planes + 3 halo each side
NOUT = 8                   # output planes per core
GP = 4                     # output planes per PSUM group
BF16 = mybir.dt.bfloat16
F32 = mybir.dt.float32

_cached = None  # (nc, input names) — compile once per process


def _build_nc(conv_repeat=1, with_collective=True):
    nc = bacc.Bacc("TRN2", target_bir_lowering=False, debug=False, num_devices=N_CORES)

    x_in = nc.dram_tensor("x_in", [NP, CIN, S, S], F32, kind="ExternalInput").ap()
    w0 = nc.dram_tensor("w0", [49, C1, K * COUT], BF16, kind="ExternalInput").ap()
    w1 = nc.dram_tensor("w1", [49, C2, K * COUT], BF16, kind="ExternalInput").ap()
    sel = nc.dram_tensor("sel", [COUT, 36], F32, kind="ExternalInput").ap()
    ind = nc.dram_tensor("ind", [4, 256], F32, kind="ExternalInput").ap()
    bias_in = nc.dram_tensor("bias_in", [16, 1], F32, kind="ExternalInput").ap()
    y_out = nc.dram_tensor("y", [COUT, NOUT, S * S], F32, kind="ExternalOutput").ap()

    cc_in = nc.dram_tensor("cc_in", [1, 4], F32)
    cc_out = nc.dram_tensor("cc_out", [1, 4], F32, addr_space="Shared")

    with tile.TileContext(nc) as tc, ExitStack() as ctx:
        xpool = ctx.enter_context(tc.tile_pool(name="x", bufs=1))
        spool = ctx.enter_context(tc.tile_pool(name="stage", bufs=3))
        stat = ctx.enter_context(tc.tile_pool(name="stat", bufs=1))
        wpool = ctx.enter_context(tc.tile_pool(name="w", bufs=3))
        opool = ctx.enter_context(tc.tile_pool(name="o", bufs=3))

        # resident bf16 activation tiles, zeroed (borders stay zero)
        X1 = [
            xpool.tile([C1, PAD, PAD], BF16, tag=f"x1_{p}", name=f"x1_{p}")
            for p in range(NP)
        ]
        X2 = [
            xpool.tile([C2, PAD, PAD], BF16, tag=f"x2_{p}", name=f"x2_{p}")
            for p in range(NP)
        ]
        for p in range(NP):
            nc.vector.memset(X1[p][:], 0.0)
            nc.vector.memset(X2[p][:], 0.0)

        selT = stat.tile([COUT, 36], F32)
        nc.sync.dma_start(selT[:], sel[:])
        R = stat.tile([36, 1], F32)
        nc.vector.memset(R[:], 0.0)
        bt = stat.tile([16, 1], F32)
        nc.sync.dma_start(bt[:], bias_in[:])

        # ---- phase A: load planes, convert to bf16, accumulate norm^2 maxima
        with tc.tile_pool(name="spsum", bufs=2, space="PSUM") as spsum:
            for p in range(NP):
                s1 = spool.tile([C1, S, S], F32, tag="s1")
                nc.sync.dma_start(s1[:], x_in[p, 0:C1])
                s2 = spool.tile([C2, S, S], F32, tag="s2")
                nc.sync.dma_start(s2[:], x_in[p, C1:CIN])
                nc.scalar.copy(X1[p][:, 3 : 3 + S, 3 : 3 + S], s1[:])
                nc.scalar.copy(X2[p][:, 3 : 3 + S, 3 : 3 + S], s2[:])
                sq = spool.tile([COUT, S, S], F32, tag="sq")
                nc.vector.tensor_tensor(
                    sq[:], s1[0:COUT], s1[0:COUT], op=mybir.AluOpType.mult
                )
                for h in range(2):
                    ps = spsum.tile([36, 512], F32, tag="sps")
                    nc.tensor.matmul(
                        ps[:], selT[:], sq[:, 16 * h : 16 * (h + 1), :],
                        start=True, stop=True,
                    )
                    tmp = spool.tile([36, 1], F32, tag="rtmp")
                    nc.vector.reduce_max(tmp[:], ps[:], axis=mybir.AxisListType.X)
                    nc.vector.tensor_tensor(R[:], R[:], tmp[:], op=mybir.AluOpType.max)

            # ---- phase B: finalize stats, AllReduce(max), build scale vectors
            RT = stat.tile([1, 36], F32)
            nc.sync.dma_start(RT[:], R[:])
            row4 = stat.tile([1, 4], F32)
            nc.vector.reduce_max(row4[0:1, 0:1], RT[0:1, 0:16], axis=mybir.AxisListType.X)
            nc.vector.reduce_max(row4[0:1, 1:2], RT[0:1, 16:32], axis=mybir.AxisListType.X)
            nc.vector.reduce_max(row4[0:1, 2:3], RT[0:1, 32:36], axis=mybir.AxisListType.X)
            nc.vector.tensor_tensor(
                row4[0:1, 3:4], row4[0:1, 1:2], row4[0:1, 1:2], op=mybir.AluOpType.mult
            )
            nc.sync.dma_start(cc_in.ap()[:], row4[:])
            if with_collective:
                nc.gpsimd.collective_compute(
                    "AllReduce", mybir.AluOpType.max,
                    replica_groups=[list(range(N_CORES))],
                    ins=[cc_in.ap()[:]], outs=[cc_out.ap()[:]],
                )
                cc_src = cc_out
            else:
                cc_src = cc_in
            g4 = stat.tile([1, 4], F32)
            nc.sync.dma_start(g4[:], cc_src.ap()[:])
            eps = stat.tile([1, 1], F32)
            nc.vector.memset(eps[:], 1e-12)
            nc.scalar.activation(g4[:], g4[:], mybir.ActivationFunctionType.Sqrt, bias=eps[:])
            nc.scalar.activation(g4[:], g4[:], mybir.ActivationFunctionType.Copy, bias=1e-5)
            nc.vector.reciprocal(g4[:], g4[:])
            g4T = stat.tile([4, 1], F32)
            nc.sync.dma_start(g4T[:], g4[:])
            indt = stat.tile([4, 256], F32)
            nc.sync.dma_start(indt[:], ind[:])
            scvA = stat.tile([C1, 1], F32)
            scvB = stat.tile([C1, 1], F32)
            for scv, off in ((scvA, 0), (scvB, 128)):
                pb = spsum.tile([C1, 1], F32, tag="pscv")
                nc.tensor.matmul(
                    pb[:], indt[:, off : off + 128], g4T[:], start=True, stop=True
                )
                nc.vector.tensor_copy(scv[:], pb[:])

        # ---- phase C: scale activations in place
        for p in range(NP):
            nc.vector.tensor_scalar_mul(X1[p][:], X1[p][:], scvA[:])
            nc.vector.tensor_scalar_mul(X2[p][:], X2[p][:], scvB[0:C2])

        # ---- phase D: 343-tap conv, 2 groups of 4 output planes
        with tc.tile_pool(name="cpsum", bufs=1, space="PSUM") as cpsum:
            for g in [grp for _ in range(conv_repeat) for grp in range(2)]:
                P = [
                    [
                        cpsum.tile(
                            [COUT, 512], F32, tag=f"p{pl}_{h}", name=f"p{g}_{pl}_{h}"
                        )
                        for h in range(2)
                    ]
                    for pl in range(GP)
                ]
                for kxy in range(49):
                    kh, kw = divmod(kxy, 7)
                    wt0 = wpool.tile([C1, K * COUT], BF16, tag="w0")
                    nc.sync.dma_start(wt0[:], w0[kxy])
                    wt1 = wpool.tile([C2, K * COUT], BF16, tag="w1")
                    nc.sync.dma_start(wt1[:], w1[kxy])
                    first = kxy == 0
                    last = kxy == 48
                    for kd in range(K):
                        # keep the stationary operand fixed across 8 MMs
                        for lhsT, X, chunk in (
                            (wt0[:, kd * COUT : (kd + 1) * COUT], X1, 0),
                            (wt1[:, kd * COUT : (kd + 1) * COUT], X2, 1),
                        ):
                            for pl in range(GP):
                                xp = g * GP + pl + kd
                                for h in range(2):
                                    rhs = X[xp][
                                        :, kh + 16 * h : kh + 16 * (h + 1), kw : kw + S
                                    ]
                                    nc.tensor.matmul(
                                        P[pl][h][:], lhsT, rhs,
                                        start=first and kd == 0 and chunk == 0,
                                        stop=last and kd == K - 1 and chunk == 1,
                                    )
                for pl in range(GP):
                    ot = opool.tile([COUT, S * S], F32, tag="ot")
                    nc.vector.tensor_copy(ot[:, 0:512], P[pl][0][:])
                    nc.vector.tensor_copy(ot[:, 512:1024], P[pl][1][:])
                    nc.scalar.activation(
                        ot[0:16, :], ot[0:16, :],
                        mybir.ActivationFunctionType.Relu, bias=bt[:],
                    )
                    nc.sync.dma_start(y_out[:, g * GP + pl, :], ot[:])

    nc.compile()
    return nc


MULS_IN = (16, 16, 4, 16)
DIMS_IN = (1, 3, 5, 9)
MULS_OUT = (16, 16, 4)
DIMS_OUT = (1, 3, 5)


def _host_prep(sv5, basis, weights, bias):
    # permuted activation volume (l1 i-major, l2 d-major, tensor-square ij-major)
    v = sv5[:, 16:64].reshape(B, 16, 3, S, S, S)
    x = np.empty((B, CIN, S, S, S), np.float32)
    x[:, 0:16] = sv5[:, 0:16]
    x[:, 16:64] = v.transpose(0, 2, 1, 3, 4, 5).reshape(B, 48, S, S, S)
    x[:, 64:84] = (
        sv5[:, 64:84].reshape(B, 4, 5, S, S, S).transpose(0, 2, 1, 3, 4, 5)
        .reshape(B, 20, S, S, S)
    )
    t = v[:, :, :, None] * v[:, :, None, :]  # [B,16,3,3,sp]
    x[:, 84:228] = t.transpose(0, 2, 3, 1, 4, 5, 6).reshape(B, 144, S, S, S)

    # assemble the steerable kernel [84, 228, 7,7,7] in reference channel order
    rows = []
    for o, (mo, do) in enumerate(zip(MULS_OUT, DIMS_OUT)):
        cols = []
        for i, (mi, di) in enumerate(zip(MULS_IN, DIMS_IN)):
            bas = basis[o, i, :, :do, :di]
            w = weights[o, i, :, :mo, :mi]
            kb = np.einsum("puv,pabxyz->uavbxyz", w, bas)
            cols.append(kb.reshape(mo * do, mi * di, K, K, K))
        rows.append(np.concatenate(cols, axis=1))
    kern = np.concatenate(rows, axis=0)

    # input-channel permutation matching x's layout
    perm = np.empty(CIN, np.int64)
    perm[0:16] = np.arange(16)
    for i in range(3):
        for m in range(16):
            perm[16 + 16 * i + m] = 16 + 3 * m + i
    for d in range(5):
        for m in range(4):
            perm[64 + 4 * d + m] = 64 + 5 * m + d
    for ij in range(9):
        for m in range(16):
            perm[84 + 16 * ij + m] = 84 + 9 * m + ij
    kern = kern[:, perm]  # [84, 228, 7, 7, 7]

    # [kh*7+kw, c, kd*84+o] bf16
    W = np.ascontiguousarray(kern.transpose(3, 4, 1, 2, 0)).reshape(49, CIN, K * COUT)
    W = W.astype(ml_dtypes.bfloat16)
    W0 = np.ascontiguousarray(W[:, 0:C1])
    W1 = np.ascontiguousarray(W[:, C1:CIN])

    selm = np.zeros((COUT, 36), np.float32)
    for m in range(16):
        selm[m, m] = 1.0
        for i in range(3):
            selm[16 + 16 * i + m, 16 + m] = 1.0
    for m in range(4):
        for d in range(5):
            selm[64 + 4 * d + m, 32 + m] = 1.0

    indm = np.zeros((4, 256), np.float32)
    for bnum, (s0, e0) in enumerate([(0, 16), (16, 64), (64, 84), (84, 128)]):
        indm[bnum, s0:e0] = 1.0
    indm[3, 128:256] = 1.0

    return x, W0, W1, selm, indm, bias.reshape(16, 1).astype(np.float32)


def kernel(sv5, basis, weights, bias):
    global _cached
    sv5 = np.asarray(sv5, np.float32)
    basis = np.asarray(basis, np.float32)
    weights = np.asarray(weights, np.float32)
    bias = np.asarray(bias, np.float32)

    x, W0, W1, selm, indm, biasm = _host_prep(sv5, basis, weights, bias)

    in_maps = []
    for c in range(N_CORES):
        bb, zi = divmod(c, 4)
        dz = zi * NOUT
        sl = np.zeros((NP, CIN, S, S), np.float32)
        for p in range(NP):
            gz = dz + p - 3
            if 0 <= gz < S:
                sl[p] = x[bb, :, gz]
        in_maps.append(
            {"x_in": sl, "w0": W0, "w1": W1, "sel": selm, "ind": indm, "bias_in": biasm}
        )

    global _last_in_maps
    _last_in_maps = in_maps
    if _cached is None:
        _cached = _build_nc()
    nc = _cached

    res = run_bass_kernel_spmd(nc, in_maps, core_ids=list(range(N_CORES)))

    out = np.empty((B, COUT, S, S, S), np.float32)
    for c in range(N_CORES):
        bb, zi = divmod(c, 4)
        dz = zi * NOUT
        out[bb, :, dz : dz + NOUT] = res.results[c]["y"].reshape(COUT, NOUT, S, S)
    return out


# revision 15
# speedup vs baseline: 1.1500x; 1.1500x over previous
"""Steerable 3D conv block (nn_Block_66795331387589) on 8 Trainium2 NeuronCores.

Pipeline per core (data-parallel over batch x D-slabs, halo sliced on host):
  1. host: tensor-square channels, channel permutation, kernel assembly
     (basis x weights einsum), per-core D-slab slicing with 3-voxel halo.
  2. device: BN max-statistics (field norms via selector matmul), 8-core
     AllReduce(max), per-block scale vector via indicator matmul, scale the
     bf16 activation tiles in place, then the 7x7x7 conv as 343-tap
     shift-and-accumulate bf16 matmuls into PSUM, bias+ReLU on the l=0
     output channels, DMA out.
"""
import sys

sys.path.insert(0, "/opt/trn_rl_repo")

from contextlib import ExitStack

import ml_dtypes
import numpy as np

import concourse.bass as bass
import concourse.tile as tile
from concourse import bacc, mybir
from concourse.bass_utils import run_bass_kernel_spmd

N_CORES = 8
B, S = 2, 32
C1, C2 = 128, 100          # contract chunks over the 228 permuted channels
CIN = 228
COUT = 84
K = 7
PAD = S + 2 * 3            # 38
NP = 14                    # 8 owned planes + 3 halo each side
NOUT = 8                   # output planes per core
GP = 4                     # output planes per PSUM group
BF16 = mybir.dt.bfloat16
F32 = mybir.dt.float32

_cached = None  # (nc, input names) — compile once per process


def _segs():
    """Per input-plane stream (xp_rel 0..9): PSUM col segments in the packed
    (d*84+o) slot space, 64-aligned starts, not crossing 128-slot banks."""
    out = []
    for xp in range(10):
        dlo, dhi = max(0, xp - 6), min(3, xp)
        a, b = dlo * 84, (dhi + 1) * 84
        s = (a // 64) * 64
        segs = []
        while s < b:
            bank = s // 128
            end = min(b, 128 * (bank + 1))
            segs.append((s, end - s, bank, s - 128 * bank, a))
            s = end
        out.append(segs)
    return out


SEGS = _segs()
CUMS = []
_c = 0
for _segs_xp in SEGS:
    _cl = []
    for (_s0, _ln, _b, _ls, _a) in _segs_xp:
        _cl.append(_c)
        _c += _ln
    CUMS.append(_cl)
NCOLS = _c
BUSED = (128, 128, 80)  # used partitions per packed PSUM bank


def _build_nc(conv_repeat=1, with_collective=True):
    nc = bacc.Bacc("TRN2", target_bir_lowering=False, debug=False, num_devices=N_CORES)

    x_in = nc.dram_tensor("x_in", [NP, CIN, S, S], F32, kind="ExternalInput").ap()
    w0 = nc.dram_tensor("w0", [49, C1, NCOLS], BF16, kind="ExternalInput").ap()
    w1 = nc.dram_tensor("w1", [49, C2, NCOLS], BF16, kind="ExternalInput").ap()
    sel = nc.dram_tensor("sel", [COUT, 36], F32, kind="ExternalInput").ap()
    ind = nc.dram_tensor("ind", [4, 256], F32, kind="ExternalInput").ap()
    bias_in = nc.dram_tensor("bias_in", [16, 1], F32, kind="ExternalInput").ap()
    y_out = nc.dram_tensor("y", [COUT, NOUT, S * S], F32, kind="ExternalOutput").ap()

    cc_in = nc.dram_tensor("cc_in", [1, 4], F32)
    cc_out = nc.dram_tensor("cc_out", [1, 4], F32, addr_space="Shared")

    with tile.TileContext(nc) as tc, ExitStack() as ctx:
        xpool = ctx.enter_context(tc.tile_pool(name="x", bufs=1))
        spool = ctx.enter_context(tc.tile_pool(name="stage", bufs=3))
        stat = ctx.enter_context(tc.tile_pool(name="stat", bufs=1))
        wpool = ctx.enter_context(tc.tile_pool(name="w", bufs=3))
        opool = ctx.enter_context(tc.tile_pool(name="o", bufs=3))

        # resident bf16 activation tiles, zeroed (borders stay zero)
        X1 = [
            xpool.tile([C1, PAD, PAD], BF16, tag=f"x1_{p}", name=f"x1_{p}")
            for p in range(NP)
        ]
        X2 = [
            xpool.tile([C2, PAD, PAD], BF16, tag=f"x2_{p}", name=f"x2_{p}")
            for p in range(NP)
        ]
        for p in range(NP):
            nc.vector.memset(X1[p][:], 0.0)
            nc.vector.memset(X2[p][:], 0.0)

        selT = stat.tile([COUT, 36], F32)
        nc.sync.dma_start(selT[:], sel[:])
        R = stat.tile([36, 1], F32)
        nc.vector.memset(R[:], 0.0)
        bt = stat.tile([16, 1], F32)
        nc.sync.dma_start(bt[:], bias_in[:])

        # ---- phase A: load planes, convert to bf16, accumulate norm^2 maxima
        with tc.tile_pool(name="spsum", bufs=2, space="PSUM") as spsum:
            for p in range(NP):
                s1 = spool.tile([C1, S, S], F32, tag="s1")
                nc.sync.dma_start(s1[:], x_in[p, 0:C1])
                s2 = spool.tile([C2, S, S], F32, tag="s2")
                nc.sync.dma_start(s2[:], x_in[p, C1:CIN])
                nc.scalar.copy(X1[p][:, 3 : 3 + S, 3 : 3 + S], s1[:])
                nc.scalar.copy(X2[p][:, 3 : 3 + S, 3 : 3 + S], s2[:])
                sq = spool.tile([COUT, S, S], F32, tag="sq")
                nc.vector.tensor_tensor(
                    sq[:], s1[0:COUT], s1[0:COUT], op=mybir.AluOpType.mult
                )
                for h in range(2):
                    ps = spsum.tile([36, 512], F32, tag="sps")
                    nc.tensor.matmul(
                        ps[:], selT[:], sq[:, 16 * h : 16 * (h + 1), :],
                        start=True, stop=True,
                    )
                    tmp = spool.tile([36, 1], F32, tag="rtmp")
                    nc.vector.reduce_max(tmp[:], ps[:], axis=mybir.AxisListType.X)
                    nc.vector.tensor_tensor(R[:], R[:], tmp[:], op=mybir.AluOpType.max)

            # ---- phase B: finalize stats, AllReduce(max), build scale vectors
            RT = stat.tile([1, 36], F32)
            nc.sync.dma_start(RT[:], R[:])
            row4 = stat.tile([1, 4], F32)
            nc.vector.reduce_max(row4[0:1, 0:1], RT[0:1, 0:16], axis=mybir.AxisListType.X)
            nc.vector.reduce_max(row4[0:1, 1:2], RT[0:1, 16:32], axis=mybir.AxisListType.X)
            nc.vector.reduce_max(row4[0:1, 2:3], RT[0:1, 32:36], axis=mybir.AxisListType.X)
            nc.vector.tensor_tensor(
                row4[0:1, 3:4], row4[0:1, 1:2], row4[0:1, 1:2], op=mybir.AluOpType.mult
            )
            nc.sync.dma_start(cc_in.ap()[:], row4[:])
            if with_collective:
                nc.gpsimd.collective_compute(
                    "AllReduce", mybir.AluOpType.max,
                    replica_groups=[list(range(N_CORES))],
                    ins=[cc_in.ap()[:]], outs=[cc_out.ap()[:]],
                )
                cc_src = cc_out
            else:
                cc_src = cc_in
            g4 = stat.tile([1, 4], F32)
            nc.sync.dma_start(g4[:], cc_src.ap()[:])
            eps = stat.tile([1, 1], F32)
            nc.vector.memset(eps[:], 1e-12)
            nc.scalar.activation(g4[:], g4[:], mybir.ActivationFunctionType.Sqrt, bias=eps[:])
            nc.scalar.activation(g4[:], g4[:], mybir.ActivationFunctionType.Copy, bias=1e-5)
            nc.vector.reciprocal(g4[:], g4[:])
            g4T = stat.tile([4, 1], F32)
            nc.sync.dma_start(g4T[:], g4[:])
            indt = stat.tile([4, 256], F32)
            nc.sync.dma_start(indt[:], ind[:])
            scvA = stat.tile([C1, 1], F32)
            scvB = stat.tile([C1, 1], F32)
            for scv, off in ((scvA, 0), (scvB, 128)):
                pb = spsum.tile([C1, 1], F32, tag="pscv")
                nc.tensor.matmul(
                    pb[:], indt[:, off : off + 128], g4T[:], start=True, stop=True
                )
                nc.vector.tensor_copy(scv[:], pb[:])

        # ---- phase C: scale activations in place
        for p in range(NP):
            nc.vector.tensor_scalar_mul(X1[p][:], X1[p][:], scvA[:])
            nc.vector.tensor_scalar_mul(X2[p][:], X2[p][:], scvB[0:C2])

        # ---- phase D: 343-tap conv, packed output columns (d*84+o slots over
        # 3 PSUM banks x 2 halves), 2 groups of 4 output planes
        with tc.tile_pool(name="cpsum", bufs=1, space="PSUM") as cpsum:
            for g in [grp for _ in range(conv_repeat) for grp in range(2)]:
                PB = [
                    [
                        cpsum.tile([128, 512], F32, tag=f"pb{h}_{b}", name=f"pb{g}{h}{b}")
                        for b in range(3)
                    ]
                    for h in range(2)
                ]
                seen = set()
                for kxy in range(49):
                    kh, kw = divmod(kxy, 7)
                    wt0 = wpool.tile([C1, NCOLS], BF16, tag="w0")
                    nc.sync.dma_start(wt0[:], w0[kxy])
                    wt1 = wpool.tile([C2, NCOLS], BF16, tag="w1")
                    nc.sync.dma_start(wt1[:], w1[kxy])
                    # xp 3 covers every bank's full used region -> it must run
                    # first so its start=True MMs initialize each bank
                    xporder = [3, 0, 1, 2, 4, 5, 6, 7, 8, 9] if kxy == 0 else range(10)
                    for wt, X, chunk in ((wt0, X1, 0), (wt1, X2, 1)):
                        for xp_rel in xporder:
                            xp = g * GP + xp_rel
                            for (s0, ln, bank, ls, a), cum in zip(
                                SEGS[xp_rel], CUMS[xp_rel]
                            ):
                                stop = (
                                    kxy == 48 and chunk == 1
                                    and ((xp_rel == 7 and bank == 0) or (xp_rel == 9 and bank != 0))
                                )
                                for h in range(2):
                                    key = (h, bank)
                                    st = key not in seen
                                    seen.add(key)
                                    rhs = X[xp][
                                        :, kh + 16 * h : kh + 16 * (h + 1), kw : kw + S
                                    ]
                                    nc.tensor.matmul(
                                        PB[h][bank][ls : ls + ln, :],
                                        wt[:, cum : cum + ln], rhs,
                                        start=st, stop=stop,
                                        tile_position=(0, 64) if ls == 64 else None,
                                        skip_group_check=True,
                                    )
                # drain: copy banks to SBUF, redistribute slots to per-plane
                # tiles via SBUF->SBUF DMA, bias+relu, DMA out
                OT = [
                    opool.tile([COUT, S * S], F32, tag=f"ot{d}", name=f"ot{g}{d}", bufs=2)
                    for d in range(GP)
                ]
                pieces = [  # (bank, psum partition, dest plane, dest channel, count)
                    (0, 0, 0, 0, 84),
                    (0, 84, 1, 0, 44), (1, 0, 1, 44, 40),
                    (1, 40, 2, 0, 84),
                    (1, 124, 3, 0, 4), (2, 0, 3, 4, 80),
                ]
                for h in range(2):
                    stages = []
                    for b in range(3):
                        stg = opool.tile(
                            [128, 512], F32, tag=f"stg{b}", name=f"stg{g}{h}{b}", bufs=2
                        )
                        nc.vector.tensor_copy(stg[0 : BUSED[b], :], PB[h][b][0 : BUSED[b], :])
                        stages.append(stg)
                    for b, p0, d, oo, ln in pieces:
                        nc.sync.dma_start(
                            OT[d][oo : oo + ln, 512 * h : 512 * (h + 1)],
                            stages[b][p0 : p0 + ln, :],
                        )
                for d in range(GP):
                    nc.scalar.activation(
                        OT[d][0:16, :], OT[d][0:16, :],
                        mybir.ActivationFunctionType.Relu, bias=bt[:],
                    )
                    nc.sync.dma_start(y_out[:, g * GP + d, :], OT[d][:])

    nc.compile()
    return nc


MULS_IN = (16, 16, 4, 16)
DIMS_IN = (1, 3, 5, 9)
MULS_OUT = (16, 16, 4)
DIMS_OUT = (1, 3, 5)


def _host_prep(sv5, basis, weights, bias):
    # permuted activation volume (l1 i-major, l2 d-major, tensor-square ij-major)
    v = sv5[:, 16:64].reshape(B, 16, 3, S, S, S)
    x = np.empty((B, CIN, S, S, S), np.float32)
    x[:, 0:16] = sv5[:, 0:16]
    x[:, 16:64] = v.transpose(0, 2, 1, 3, 4, 5).reshape(B, 48, S, S, S)
    x[:, 64:84] = (
        sv5[:, 64:84].reshape(B, 4, 5, S, S, S).transpose(0, 2, 1, 3, 4, 5)
        .reshape(B, 20, S, S, S)
    )
    t = v[:, :, :, None] * v[:, :, None, :]  # [B,16,3,3,sp]
    x[:, 84:228] = t.transpose(0, 2, 3, 1, 4, 5, 6).reshape(B, 144, S, S, S)

    # assemble the steerable kernel [84, 228, 7,7,7] in reference channel order
    rows = []
    for o, (mo, do) in enumerate(zip(MULS_OUT, DIMS_OUT)):
        cols = []
        for i, (mi, di) in enumerate(zip(MULS_IN, DIMS_IN)):
            bas = basis[o, i, :, :do, :di]
            w = weights[o, i, :, :mo, :mi]
            kb = np.einsum("puv,pabxyz->uavbxyz", w, bas)
            cols.append(kb.reshape(mo * do, mi * di, K, K, K))
        rows.append(np.concatenate(cols, axis=1))
    kern = np.concatenate(rows, axis=0)

    # input-channel permutation matching x's layout
    perm = np.empty(CIN, np.int64)
    perm[0:16] = np.arange(16)
    for i in range(3):
        for m in range(16):
            perm[16 + 16 * i + m] = 16 + 3 * m + i
    for d in range(5):
        for m in range(4):
            perm[64 + 4 * d + m] = 64 + 5 * m + d
    for ij in range(9):
        for m in range(16):
            perm[84 + 16 * ij + m] = 84 + 9 * m + ij
    kern = kern[:, perm]  # [84, 228, 7, 7, 7]

    # packed lhsT columns: per (kxy, stream xp, segment) — col j of segment
    # (s0,ln) is slot s0+j -> (d, o, kd=xp-d); slots below the stream's true
    # range get zero weights (they only add 0 into other streams' slots).
    Wp = np.zeros((49, CIN, NCOLS), np.float32)
    for kxy in range(49):
        kh, kw = divmod(kxy, 7)
        for xp in range(10):
            for (s0, ln, bank, ls, a), cum in zip(SEGS[xp], CUMS[xp]):
                slots = np.arange(s0, s0 + ln)
                vs = slots >= a
                d = slots[vs] // COUT
                o = slots[vs] % COUT
                kd = xp - d
                block = np.zeros((ln, CIN), np.float32)
                block[vs] = kern[o, :, kd, kh, kw]
                Wp[kxy, :, cum : cum + ln] = block.T
    Wp = Wp.astype(ml_dtypes.bfloat16)
    W0 = np.ascontiguousarray(Wp[:, 0:C1])
    W1 = np.ascontiguousarray(Wp[:, C1:CIN])

    selm = np.zeros((COUT, 36), np.float32)
    for m in range(16):
        selm[m, m] = 1.0
        for i in range(3):
            selm[16 + 16 * i + m, 16 + m] = 1.0
    for m in range(4):
        for d in range(5):
            selm[64 + 4 * d + m, 32 + m] = 1.0

    indm = np.zeros((4, 256), np.float32)
    for bnum, (s0, e0) in enumerate([(0, 16), (16, 64), (64, 84), (84, 128)]):
        indm[bnum, s0:e0] = 1.0
    indm[3, 128:256] = 1.0

    return x, W0, W1, selm, indm, bias.reshape(16, 1).astype(np.float32)


def kernel(sv5, basis, weights, bias):
    global _cached
    sv5 = np.asarray(sv5, np.float32)
    basis = np.asarray(basis, np.float32)
    weights = np.asarray(weights, np.float32)
    bias = np.asarray(bias, np.float32)

    x, W0, W1, selm, indm, biasm = _host_prep(sv5, basis, weights, bias)

    in_maps = []
    for c in range(N_CORES):
        bb, zi = divmod(c, 4)
        dz = zi * NOUT
        sl = np.zeros((NP, CIN, S, S), np.float32)
        for p in range(NP):
            gz = dz + p - 3
            if 0 <= gz < S:
                sl[p] = x[bb, :, gz]
        in_maps.append(
            {"x_in": sl, "w0": W0, "w1": W1, "sel": selm, "ind": indm, "bias_in": biasm}
        )

    global _last_in_maps
    _last_in_maps = in_maps
    if _cached is None:
        _cached = _build_nc()
    nc = _cached

    res = run_bass_kernel_spmd(nc, in_maps, core_ids=list(range(N_CORES)))

    out = np.empty((B, COUT, S, S, S), np.float32)
    for c in range(N_CORES):
        bb, zi = divmod(c, 4)
        dz = zi * NOUT
        out[bb, :, dz : dz + NOUT] = res.results[c]["y"].reshape(COUT, NOUT, S, S)
    return out


# revision 16
# speedup vs baseline: 1.1636x; 1.0118x over previous
"""Steerable 3D conv block (nn_Block_66795331387589) on 8 Trainium2 NeuronCores.

Pipeline per core (data-parallel over batch x D-slabs, halo sliced on host):
  1. host: tensor-square channels, channel permutation, kernel assembly
     (basis x weights einsum), per-core D-slab slicing with 3-voxel halo.
  2. device: BN max-statistics (field norms via selector matmul), 8-core
     AllReduce(max), per-block scale vector via indicator matmul, scale the
     bf16 activation tiles in place, then the 7x7x7 conv as 343-tap
     shift-and-accumulate bf16 matmuls into PSUM, bias+ReLU on the l=0
     output channels, DMA out.
"""
import sys

sys.path.insert(0, "/opt/trn_rl_repo")

from contextlib import ExitStack

import ml_dtypes
import numpy as np

import concourse.bass as bass
import concourse.tile as tile
from concourse import bacc, mybir
from concourse.bass_utils import run_bass_kernel_spmd

N_CORES = 8
B, S = 2, 32
C1, C2 = 128, 100          # contract chunks over the 228 permuted channels
CIN = 228
COUT = 84
K = 7
PAD = S + 2 * 3            # 38
NP = 14                    # 8 owned planes + 3 halo each side
NOUT = 8                   # output planes per core
GP = 4                     # output planes per PSUM group
BF16 = mybir.dt.bfloat16
F32 = mybir.dt.float32

_cached = None  # (nc, input names) — compile once per process


def _segs():
    """Per input-plane stream (xp_rel 0..9): PSUM col segments in the packed
    (d*84+o) slot space, 64-aligned starts, not crossing 128-slot banks."""
    out = []
    for xp in range(10):
        dlo, dhi = max(0, xp - 6), min(3, xp)
        a, b = dlo * 84, (dhi + 1) * 84
        s = (a // 64) * 64
        segs = []
        while s < b:
            bank = s // 128
            end = min(b, 128 * (bank + 1))
            segs.append((s, end - s, bank, s - 128 * bank, a))
            s = end
        out.append(segs)
    return out


SEGS = _segs()
CUMS = []
_c = 0
for _segs_xp in SEGS:
    _cl = []
    for (_s0, _ln, _b, _ls, _a) in _segs_xp:
        _cl.append(_c)
        _c += _ln
    CUMS.append(_cl)
NCOLS = _c
BUSED = (128, 128, 80)  # used partitions per packed PSUM bank


def _build_nc(conv_repeat=1, with_collective=True):
    nc = bacc.Bacc("TRN2", target_bir_lowering=False, debug=False, num_devices=N_CORES)

    x_in = nc.dram_tensor("x_in", [NP, CIN, S, S], F32, kind="ExternalInput").ap()
    w0 = nc.dram_tensor("w0", [49, C1, NCOLS], BF16, kind="ExternalInput").ap()
    w1 = nc.dram_tensor("w1", [49, C2, NCOLS], BF16, kind="ExternalInput").ap()
    sel = nc.dram_tensor("sel", [COUT, 36], F32, kind="ExternalInput").ap()
    ind = nc.dram_tensor("ind", [4, 256], F32, kind="ExternalInput").ap()
    bias_in = nc.dram_tensor("bias_in", [16, 1], F32, kind="ExternalInput").ap()
    y_out = nc.dram_tensor("y", [COUT, NOUT, S * S], F32, kind="ExternalOutput").ap()

    cc_in = nc.dram_tensor("cc_in", [1, 4], F32)
    cc_out = nc.dram_tensor("cc_out", [1, 4], F32, addr_space="Shared")

    with tile.TileContext(nc) as tc, ExitStack() as ctx:
        xpool = ctx.enter_context(tc.tile_pool(name="x", bufs=1))
        spool = ctx.enter_context(tc.tile_pool(name="stage", bufs=3))
        stat = ctx.enter_context(tc.tile_pool(name="stat", bufs=1))
        wpool = ctx.enter_context(tc.tile_pool(name="w", bufs=3))
        opool = ctx.enter_context(tc.tile_pool(name="o", bufs=3))

        # resident bf16 activation tiles, zeroed (borders stay zero)
        X1 = [
            xpool.tile([C1, PAD, PAD], BF16, tag=f"x1_{p}", name=f"x1_{p}")
            for p in range(NP)
        ]
        X2 = [
            xpool.tile([C2, PAD, PAD], BF16, tag=f"x2_{p}", name=f"x2_{p}")
            for p in range(NP)
        ]
        # only the 3-voxel borders need zeroing; the interior is fully
        # overwritten by the fp32->bf16 converts in phase A
        for p in range(NP):
            for xt in (X1[p], X2[p]):
                nc.vector.memset(xt[:, 0:3, :], 0.0)
                nc.vector.memset(xt[:, 35:38, :], 0.0)
                nc.vector.memset(xt[:, 3:35, 0:3], 0.0)
                nc.vector.memset(xt[:, 3:35, 35:38], 0.0)

        selT = stat.tile([COUT, 36], F32)
        nc.sync.dma_start(selT[:], sel[:])
        R = stat.tile([36, 1], F32)
        nc.vector.memset(R[:], 0.0)
        bt = stat.tile([16, 1], F32)
        nc.sync.dma_start(bt[:], bias_in[:])

        # ---- phase A: load planes, convert to bf16, accumulate norm^2 maxima
        with tc.tile_pool(name="spsum", bufs=2, space="PSUM") as spsum:
            for p in range(NP):
                s1 = spool.tile([C1, S, S], F32, tag="s1")
                nc.sync.dma_start(s1[:], x_in[p, 0:C1])
                s2 = spool.tile([C2, S, S], F32, tag="s2")
                nc.sync.dma_start(s2[:], x_in[p, C1:CIN])
                nc.scalar.copy(X1[p][:, 3 : 3 + S, 3 : 3 + S], s1[:])
                nc.scalar.copy(X2[p][:, 3 : 3 + S, 3 : 3 + S], s2[:])
                sq = spool.tile([COUT, S, S], F32, tag="sq")
                nc.vector.tensor_tensor(
                    sq[:], s1[0:COUT], s1[0:COUT], op=mybir.AluOpType.mult
                )
                for h in range(2):
                    ps = spsum.tile([36, 512], F32, tag="sps")
                    nc.tensor.matmul(
                        ps[:], selT[:], sq[:, 16 * h : 16 * (h + 1), :],
                        start=True, stop=True,
                    )
                    tmp = spool.tile([36, 1], F32, tag="rtmp")
                    nc.vector.reduce_max(tmp[:], ps[:], axis=mybir.AxisListType.X)
                    nc.vector.tensor_tensor(R[:], R[:], tmp[:], op=mybir.AluOpType.max)

            # ---- phase B: finalize stats, AllReduce(max), build scale vectors
            RT = stat.tile([1, 36], F32)
            nc.sync.dma_start(RT[:], R[:])
            row4 = stat.tile([1, 4], F32)
            nc.vector.reduce_max(row4[0:1, 0:1], RT[0:1, 0:16], axis=mybir.AxisListType.X)
            nc.vector.reduce_max(row4[0:1, 1:2], RT[0:1, 16:32], axis=mybir.AxisListType.X)
            nc.vector.reduce_max(row4[0:1, 2:3], RT[0:1, 32:36], axis=mybir.AxisListType.X)
            nc.vector.tensor_tensor(
                row4[0:1, 3:4], row4[0:1, 1:2], row4[0:1, 1:2], op=mybir.AluOpType.mult
            )
            nc.sync.dma_start(cc_in.ap()[:], row4[:])
            if with_collective:
                nc.gpsimd.collective_compute(
                    "AllReduce", mybir.AluOpType.max,
                    replica_groups=[list(range(N_CORES))],
                    ins=[cc_in.ap()[:]], outs=[cc_out.ap()[:]],
                )
                cc_src = cc_out
            else:
                cc_src = cc_in
            g4 = stat.tile([1, 4], F32)
            nc.sync.dma_start(g4[:], cc_src.ap()[:])
            eps = stat.tile([1, 1], F32)
            nc.vector.memset(eps[:], 1e-12)
            nc.scalar.activation(g4[:], g4[:], mybir.ActivationFunctionType.Sqrt, bias=eps[:])
            nc.scalar.activation(g4[:], g4[:], mybir.ActivationFunctionType.Copy, bias=1e-5)
            nc.vector.reciprocal(g4[:], g4[:])
            g4T = stat.tile([4, 1], F32)
            nc.sync.dma_start(g4T[:], g4[:])
            indt = stat.tile([4, 256], F32)
            nc.sync.dma_start(indt[:], ind[:])
            scvA = stat.tile([C1, 1], F32)
            scvB = stat.tile([C1, 1], F32)
            for scv, off in ((scvA, 0), (scvB, 128)):
                pb = spsum.tile([C1, 1], F32, tag="pscv")
                nc.tensor.matmul(
                    pb[:], indt[:, off : off + 128], g4T[:], start=True, stop=True
                )
                nc.vector.tensor_copy(scv[:], pb[:])

        # ---- phase C: scale activations in place
        for p in range(NP):
            nc.vector.tensor_scalar_mul(X1[p][:], X1[p][:], scvA[:])
            nc.vector.tensor_scalar_mul(X2[p][:], X2[p][:], scvB[0:C2])

        # ---- phase D: 343-tap conv, packed output columns (d*84+o slots over
        # 3 PSUM banks x 2 halves), 2 groups of 4 output planes
        with tc.tile_pool(name="cpsum", bufs=1, space="PSUM") as cpsum:
            for g in [grp for _ in range(conv_repeat) for grp in range(2)]:
                PB = [
                    [
                        cpsum.tile([128, 512], F32, tag=f"pb{h}_{b}", name=f"pb{g}{h}{b}")
                        for b in range(3)
                    ]
                    for h in range(2)
                ]
                seen = set()
                for kxy in range(49):
                    kh, kw = divmod(kxy, 7)
                    wt0 = wpool.tile([C1, NCOLS], BF16, tag="w0")
                    nc.sync.dma_start(wt0[:], w0[kxy])
                    wt1 = wpool.tile([C2, NCOLS], BF16, tag="w1")
                    nc.sync.dma_start(wt1[:], w1[kxy])
                    # xp 3 covers every bank's full used region -> it must run
                    # first so its start=True MMs initialize each bank
                    xporder = [3, 0, 1, 2, 4, 5, 6, 7, 8, 9] if kxy == 0 else range(10)
                    for wt, X, chunk in ((wt0, X1, 0), (wt1, X2, 1)):
                        for xp_rel in xporder:
                            xp = g * GP + xp_rel
                            for (s0, ln, bank, ls, a), cum in zip(
                                SEGS[xp_rel], CUMS[xp_rel]
                            ):
                                stop = (
                                    kxy == 48 and chunk == 1
                                    and ((xp_rel == 7 and bank == 0) or (xp_rel == 9 and bank != 0))
                                )
                                for h in range(2):
                                    key = (h, bank)
                                    st = key not in seen
                                    seen.add(key)
                                    rhs = X[xp][
                                        :, kh + 16 * h : kh + 16 * (h + 1), kw : kw + S
                                    ]
                                    nc.tensor.matmul(
                                        PB[h][bank][ls : ls + ln, :],
                                        wt[:, cum : cum + ln], rhs,
                                        start=st, stop=stop,
                                        tile_position=(0, 64) if ls == 64 else None,
                                        skip_group_check=True,
                                    )
                # drain: copy banks to SBUF, redistribute slots to per-plane
                # tiles via SBUF->SBUF DMA, bias+relu, DMA out
                OT = [
                    opool.tile([COUT, S * S], F32, tag=f"ot{d}", name=f"ot{g}{d}", bufs=2)
                    for d in range(GP)
                ]
                pieces = [  # (bank, psum partition, dest plane, dest channel, count)
                    (0, 0, 0, 0, 84),
                    (0, 84, 1, 0, 44), (1, 0, 1, 44, 40),
                    (1, 40, 2, 0, 84),
                    (1, 124, 3, 0, 4), (2, 0, 3, 4, 80),
                ]
                for h in range(2):
                    stages = []
                    for b in range(3):
                        stg = opool.tile(
                            [128, 512], F32, tag=f"stg{b}", name=f"stg{g}{h}{b}", bufs=2
                        )
                        nc.vector.tensor_copy(stg[0 : BUSED[b], :], PB[h][b][0 : BUSED[b], :])
                        stages.append(stg)
                    for b, p0, d, oo, ln in pieces:
                        nc.sync.dma_start(
                            OT[d][oo : oo + ln, 512 * h : 512 * (h + 1)],
                            stages[b][p0 : p0 + ln, :],
                        )
                for d in range(GP):
                    nc.scalar.activation(
                        OT[d][0:16, :], OT[d][0:16, :],
                        mybir.ActivationFunctionType.Relu, bias=bt[:],
                    )
                    nc.sync.dma_start(y_out[:, g * GP + d, :], OT[d][:])

    nc.compile()
    return nc


MULS_IN = (16, 16, 4, 16)
DIMS_IN = (1, 3, 5, 9)
MULS_OUT = (16, 16, 4)
DIMS_OUT = (1, 3, 5)


def _host_prep(sv5, basis, weights, bias):
    # permuted activation volume (l1 i-major, l2 d-major, tensor-square ij-major)
    v = sv5[:, 16:64].reshape(B, 16, 3, S, S, S)
    x = np.empty((B, CIN, S, S, S), np.float32)
    x[:, 0:16] = sv5[:, 0:16]
    x[:, 16:64] = v.transpose(0, 2, 1, 3, 4, 5).reshape(B, 48, S, S, S)
    x[:, 64:84] = (
        sv5[:, 64:84].reshape(B, 4, 5, S, S, S).transpose(0, 2, 1, 3, 4, 5)
        .reshape(B, 20, S, S, S)
    )
    t = v[:, :, :, None] * v[:, :, None, :]  # [B,16,3,3,sp]
    x[:, 84:228] = t.transpose(0, 2, 3, 1, 4, 5, 6).reshape(B, 144, S, S, S)

    # assemble the steerable kernel [84, 228, 7,7,7] in reference channel order
    rows = []
    for o, (mo, do) in enumerate(zip(MULS_OUT, DIMS_OUT)):
        cols = []
        for i, (mi, di) in enumerate(zip(MULS_IN, DIMS_IN)):
            bas = basis[o, i, :, :do, :di]
            w = weights[o, i, :, :mo, :mi]
            kb = np.einsum("puv,pabxyz->uavbxyz", w, bas)
            cols.append(kb.reshape(mo * do, mi * di, K, K, K))
        rows.append(np.concatenate(cols, axis=1))
    kern = np.concatenate(rows, axis=0)

    # input-channel permutation matching x's layout
    perm = np.empty(CIN, np.int64)
    perm[0:16] = np.arange(16)
    for i in range(3):
        for m in range(16):
            perm[16 + 16 * i + m] = 16 + 3 * m + i
    for d in range(5):
        for m in range(4):
            perm[64 + 4 * d + m] = 64 + 5 * m + d
    for ij in range(9):
        for m in range(16):
            perm[84 + 16 * ij + m] = 84 + 9 * m + ij
    kern = kern[:, perm]  # [84, 228, 7, 7, 7]

    # packed lhsT columns: per (kxy, stream xp, segment) — col j of segment
    # (s0,ln) is slot s0+j -> (d, o, kd=xp-d); slots below the stream's true
    # range get zero weights (they only add 0 into other streams' slots).
    Wp = np.zeros((49, CIN, NCOLS), np.float32)
    for kxy in range(49):
        kh, kw = divmod(kxy, 7)
        for xp in range(10):
            for (s0, ln, bank, ls, a), cum in zip(SEGS[xp], CUMS[xp]):
                slots = np.arange(s0, s0 + ln)
                vs = slots >= a
                d = slots[vs] // COUT
                o = slots[vs] % COUT
                kd = xp - d
                block = np.zeros((ln, CIN), np.float32)
                block[vs] = kern[o, :, kd, kh, kw]
                Wp[kxy, :, cum : cum + ln] = block.T
    Wp = Wp.astype(ml_dtypes.bfloat16)
    W0 = np.ascontiguousarray(Wp[:, 0:C1])
    W1 = np.ascontiguousarray(Wp[:, C1:CIN])

    selm = np.zeros((COUT, 36), np.float32)
    for m in range(16):
        selm[m, m] = 1.0
        for i in range(3):
            selm[16 + 16 * i + m, 16 + m] = 1.0
    for m in range(4):
        for d in range(5):
            selm[64 + 4 * d + m, 32 + m] = 1.0

    indm = np.zeros((4, 256), np.float32)
    for bnum, (s0, e0) in enumerate([(0, 16), (16, 64), (64, 84), (84, 128)]):
        indm[bnum, s0:e0] = 1.0
    indm[3, 128:256] = 1.0

    return x, W0, W1, selm, indm, bias.reshape(16, 1).astype(np.float32)


def kernel(sv5, basis, weights, bias):
    global _cached
    sv5 = np.asarray(sv5, np.float32)
    basis = np.asarray(basis, np.float32)
    weights = np.asarray(weights, np.float32)
    bias = np.asarray(bias, np.float32)

    x, W0, W1, selm, indm, biasm = _host_prep(sv5, basis, weights, bias)

    in_maps = []
    for c in range(N_CORES):
        bb, zi = divmod(c, 4)
        dz = zi * NOUT
        sl = np.zeros((NP, CIN, S, S), np.float32)
        for p in range(NP):
            gz = dz + p - 3
            if 0 <= gz < S:
                sl[p] = x[bb, :, gz]
        in_maps.append(
            {"x_in": sl, "w0": W0, "w1": W1, "sel": selm, "ind": indm, "bias_in": biasm}
        )

    global _last_in_maps
    _last_in_maps = in_maps
    if _cached is None:
        _cached = _build_nc()
    nc = _cached

    res = run_bass_kernel_spmd(nc, in_maps, core_ids=list(range(N_CORES)))

    out = np.empty((B, COUT, S, S, S), np.float32)
    for c in range(N_CORES):
        bb, zi = divmod(c, 4)
        dz = zi * NOUT
        out[bb, :, dz : dz + NOUT] = res.results[c]["y"].reshape(COUT, NOUT, S, S)
    return out
